# revision 23
# baseline (speedup 1.0000x reference)
"""Competitive binding equilibrium solver on 8 TRN2 NeuronCores.

  AF = AT / (1 + K @ BF);  BF = BT / (1 + K^T @ AF)   (fixed-point sweeps)
  C  = K * AF[:, None] * BF[None, :]

Strategy: shard K row-wise (512 rows/core). Keep the local K shard SBUF-resident
in BOTH layouts (K: [i-part, j-free] and K^T: [j-part, i-free]) so each matvec
pass streams stationary tiles from SBUF. Two key optimizations over the plain
Gauss-Seidel/bf16 version:

1. fp8 weights: K tiles are stored as fp8e4m3 scaled by 64 (so K*64 sits in
   e4m3's normal range). LDWEIGHTS is the bottleneck of a matvec (the moving
   operand is a single column), and FWL loads fp8 weights 4 elems/cycle vs
   2 for bf16 - halving the per-tile cost. The moving vectors stay bf16
   (mixed-dtype matmul). Accuracy: rel err ~7e-4 (vs 2e-2 budget) since the
   reference trajectory is fully converged at 100 iters.

2. Pipelined (Jacobi) collective: the BF update at iteration i uses the
   all-gathered K^T@AF partial from iteration i-1, so the per-iteration
   AllGather overlaps an entire iteration of PE work instead of serializing.
   This is plain Jacobi iteration (both updates read the previous state); it
   converges to the same fixed point, ~2x slower per sweep, but the reference
   is converged so only the fixed point matters. rel err ~2.7e-3 at n=100.
"""

import sys

if "/opt/trn_rl_repo" not in sys.path:
    sys.path.insert(0, "/opt/trn_rl_repo")

import numpy as np

import concourse.bass as bass
import concourse.mybir as mybir
import concourse.tile as tile
from concourse import bacc
from concourse import bass_utils
from concourse.bass import ds, ts
from concourse.masks import make_identity

F32 = mybir.dt.float32
BF16 = mybir.dt.bfloat16
FP8 = mybir.dt.float8e4
ADD = mybir.AluOpType.add
MULT = mybir.AluOpType.mult
BYPASS = mybir.AluOpType.bypass

NA, NB = 4096, 4096
NCORES = 8
R = NA // NCORES          # 512 local rows per core
RT = R // 128             # 4 local row tiles (it)
JT = NB // 128            # 32 j tiles (jc)
N_ITERS = 100


def build_program(n_iters: int = N_ITERS, variant: str = "main", wdt=FP8,
                  wscale: float = 64.0, mdt=FP8, mscale: float = 64.0):
    nc = bacc.Bacc(
        "TRN2",
        target_bir_lowering=False,
        debug=False,
        num_devices=NCORES,
    )

    K_d = nc.dram_tensor("K", [R, NB], F32, kind="ExternalInput").ap()
    AT_d = nc.dram_tensor("AT", [R], F32, kind="ExternalInput").ap()
    BT_d = nc.dram_tensor("BT", [NB], F32, kind="ExternalInput").ap()
    C_d = nc.dram_tensor("C", [R, NB], F32, kind="ExternalOutput").ap()

    with tile.TileContext(nc) as tc:
        _body(tc, nc, K_d, AT_d, BT_d, C_d, n_iters, variant, wdt, wscale,
              mdt, mscale)

    nc.compile()
    return nc


def _body(tc, nc, K_d, AT_d, BT_d, C_d, n_iters, variant, wdt, wscale, mdt,
          mscale):
    rg = [list(range(NCORES))]
    S = float(wscale * mscale)    # scale of the PSUM matvec results
    assert n_iters % 2 == 0 and n_iters >= 4, "pair-batched collective"

    def P(pool, shape, dtype, tag, **kw):
        return pool.tile(shape, dtype, name=tag, tag=tag, **kw)

    from contextlib import ExitStack

    es = ExitStack()
    persist = es.enter_context(tc.tile_pool(name="persist", bufs=1))
    psum_pool = es.enter_context(tc.tile_pool(name="psum", bufs=1, space="PSUM"))
    dram_pool = es.enter_context(tc.tile_pool(name="dram", bufs=1, space="DRAM"))

    # ---- persistent SBUF tensors -------------------------------------------
    k_sb = P(persist, [128, RT, NB], wdt, "k_sb")         # wscale*K [i-part, it, j]
    kt_sb = P(persist, [128, JT, R], wdt, "kt_sb")        # wscale*K^T [j-part, jc, i]
    at_sb = P(persist, [128, RT], F32, "at_sb")           # AT[it*128+p]
    bt_sb = P(persist, [128, JT], F32, "bt_sb")           # BT[jc*128+p]
    atS = P(persist, [128, RT], F32, "atS")               # S*AT
    btS = P(persist, [128, JT], F32, "btS")               # S*BT
    atSM = P(persist, [128, RT], F32, "atSM")             # S*mscale*AT
    btSM = P(persist, [128, JT], F32, "btSM")             # S*mscale*BT
    # per-it AF moving columns: separate tiles so pass Z's it-group only
    # depends on its own column's epilogue, not the whole AF update
    af_movs_t = [P(persist, [128, 1], mdt, f"af_mov{it}") for it in range(RT)]
    af_f = P(persist, [128, RT], F32, "af_f")
    t_rt = P(persist, [128, RT], F32, "t_rt")
    bf_f = P(persist, [128, JT], F32, "bf_f")
    bf_movs = [P(persist, [128, JT], mdt, f"bf_mov{p}") for p in range(2)]
    zsum = P(persist, [128, JT], F32, "zsum")
    # two consecutive iterations' z partials ship in ONE AllGather (the ncfw
    # collective cost is latency/desc-gen dominated, ~12us regardless of
    # payload, and does not overlap compute - so halve its count)
    t_jt2 = P(persist, [128, 2, JT], F32, "t_jt2")
    zg_sbs = [P(persist, [128, NCORES, 2, JT], F32, f"zg_sb{p}") for p in range(2)]
    ident_bf = P(persist, [128, 128], BF16, "ident_bf")
    ident_f32 = P(persist, [128, 128], F32, "ident_f32")
    atbt_row = P(persist, [JT, 128], F32, "atbt_row")
    bf_row = P(persist, [JT, 128], F32, "bf_row")
    bf_flat = P(persist, [1, NB], F32, "bf_flat")
    bf_bc = P(persist, [128, NB], F32, "bf_bc")

    # ---- PSUM tensors -------------------------------------------------------
    # per-it y columns in separate tiles: group it+1's writes must not be
    # serialized behind the epilogue read of group it
    y_pss = [P(psum_pool, [128, 1], F32, f"y_ps{it}") for it in range(RT)]
    z_ps = P(psum_pool, [128, JT], F32, "z_ps")
    tr_ps = P(psum_pool, [128, 128], F32, "tr_ps")
    tr_ps_bf = P(psum_pool, [128, 128], BF16, "tr_ps_bf")

    # ---- DRAM bounce buffers for the collective (one per iteration PAIR) ---
    n_g = (n_iters + 1) // 2
    zins = [P(dram_pool, [128, 2 * JT], F32, f"zin{g}") for g in range(n_g)]
    zgathers = [
        P(dram_pool, [128 * NCORES, 2 * JT], F32, f"zgather{g}", addr_space="Shared")
        for g in range(n_g)
    ]
    bf_dram = P(dram_pool, [JT, 128], F32, "bf_dram")

    # ---- setup: identities --------------------------------------------------
    make_identity(nc, ident_bf[:])
    make_identity(nc, ident_f32[:])

    # ---- setup: AT [512] -> at_sb [128, 4]  (p, it) = AT[it*128+p] ----------
    nc.sync.dma_start(atbt_row[0:RT, :], AT_d.rearrange("(t p) -> t p", t=RT))
    nc.tensor.transpose(tr_ps[0:128, 0:RT], atbt_row[0:RT, :], ident_f32[0:RT, 0:RT])
    nc.vector.tensor_copy(at_sb[:], tr_ps[0:128, 0:RT])

    # ---- setup: BT [4096] -> bt_sb [128, 32]  (p, jc) = BT[jc*128+p] --------
    nc.sync.dma_start(atbt_row[:, :], BT_d.rearrange("(t p) -> t p", t=JT))
    nc.tensor.transpose(tr_ps[0:128, 0:JT], atbt_row[:, :], ident_f32[0:JT, 0:JT])
    nc.vector.tensor_copy(bt_sb[:], tr_ps[0:128, 0:JT])

    # ---- setup: scaled AT/BT, initial moving vectors ------------------------
    nc.vector.tensor_scalar_mul(atS[:], at_sb[:], S)
    nc.vector.tensor_scalar_mul(btS[:], bt_sb[:], S)
    nc.vector.tensor_scalar_mul(atSM[:], at_sb[:], S * mscale)
    nc.vector.tensor_scalar_mul(btSM[:], bt_sb[:], S * mscale)
    for p in range(2):
        nc.vector.tensor_scalar_mul(bf_movs[p][:], bt_sb[:], mscale)
        nc.vector.memset(zg_sbs[p][:], 0.0)
    for it in range(RT):
        nc.vector.tensor_scalar_mul(
            af_movs_t[it][:], at_sb[:, ds(it, 1)], mscale
        )

    # ---- setup: K -> k_sb (scaled cast), then PE-transpose into kt_sb -------
    with tc.tile_pool(name="stage", bufs=2) as stage_pool:
        # K rows: f32 stage -> bf16 *wscale -> (a) fp8 k_sb, (b) PE-transpose
        # (bf16; fp8 transpose is rejected by walrus) -> fp8 kt_sb
        for it in range(RT):
            stg = stage_pool.tile([128, NB], F32, tag="stage")
            nc.sync.dma_start(stg[:], K_d[ts(it, 128), :])
            k16 = stage_pool.tile([128, NB], BF16, tag="k16")
            nc.vector.tensor_scalar_mul(k16[:], stg[:], float(wscale))
            nc.vector.tensor_copy(k_sb[:, it, :], k16[:])
            for jc in range(JT):
                nc.tensor.transpose(
                    tr_ps_bf[:, :],
                    k16[:, ds(jc * 128, 128)],
                    ident_bf[:, :],
                )
                nc.vector.tensor_copy(kt_sb[:, jc, ts(it, 128)], tr_ps_bf[:, :])

        # ---- main fixed-point loop (fully unrolled; collectives cannot be in
        # control flow) -------------------------------------------------------
        for i in range(n_iters):
            bf_prev = bf_movs[(i + 1) % 2]

            # pass Y, it-major: finish each y column then run its AF epilogue
            # on the DVE while the PE streams the next column's tiles - the
            # pass Z it-group only waits on its own af_movs_t column, so the
            # PE never stalls between passes
            for it in range(RT):
                for jc in range(JT):
                    nc.tensor.matmul(
                        y_pss[it][:],
                        kt_sb[:, jc, ts(it, 128)],
                        bf_prev[:, ds(jc, 1)],
                        start=(jc == 0),
                        stop=(jc == JT - 1),
                    )
                # AF col = mscale*AF = recip(y+S)*atSM
                if variant != "pe_only":
                    nc.vector.tensor_scalar_add(
                        t_rt[:, ds(it, 1)], y_pss[it][:], S
                    )
                    nc.vector.reciprocal(t_rt[:, ds(it, 1)], t_rt[:, ds(it, 1)])
                    nc.vector.tensor_tensor(
                        af_movs_t[it][:], t_rt[:, ds(it, 1)],
                        atSM[:, ds(it, 1)], MULT,
                    )

            # pass Z: z_ps = (wscale*K)^T @ (mscale*AF) = S*z_partial
            for it in range(RT):
                for jc in range(JT):
                    nc.tensor.matmul(
                        z_ps[:, ds(jc, 1)],
                        k_sb[:, it, ds(jc * 128, 128)],
                        af_movs_t[it][:],
                        start=(it == 0 and jc == 0),
                        stop=(it == RT - 1 and jc == JT - 1),
                    )

            if variant == "pe_only":
                continue

            # stash this iteration's partial in its half of the pair buffer;
            # ship both halves in one AllGather after odd iterations
            nc.vector.tensor_copy(t_jt2[:, i % 2, :], z_ps[:])
            if i % 2 == 1:
                g = i // 2
                nc.sync.dma_start(zins[g][:], t_jt2[:])
                nc.gpsimd.collective_compute(
                    "AllGather",
                    BYPASS,
                    replica_groups=rg,
                    ins=[zins[g][:].opt()],
                    outs=[zgathers[g][:].opt()],
                )
                nc.sync.dma_start(
                    zg_sbs[g % 2][:],
                    zgathers[g][:].rearrange("(s p) c -> p s c", s=NCORES),
                )

            # BF update from z_{i-2} (two-iteration staleness)
            m = i - 2
            if m >= 0:
                g, h = m // 2, m % 2
                zg = zg_sbs[g % 2]
                nc.vector.tensor_tensor(
                    zg[:, 0:4, h, :], zg[:, 0:4, h, :], zg[:, 4:8, h, :], ADD
                )
                nc.vector.tensor_tensor(
                    zg[:, 0:2, h, :], zg[:, 0:2, h, :], zg[:, 2:4, h, :], ADD
                )
                nc.vector.scalar_tensor_tensor(
                    zsum[:], zg[:, 0, h, :], S, zg[:, 1, h, :], ADD, ADD
                )
                nc.vector.reciprocal(zsum[:], zsum[:])
                nc.vector.tensor_tensor(bf_movs[i % 2][:], zsum[:], btSM[:], MULT)

        # ---- final: BF from the freshest z (half 1 of the last gather) ------
        if variant == "pe_only":
            nc.vector.memset(bf_f[:], 1.0)
        else:
            # the loop consumed halves up to z_{n-3} (gather (n-4)/2); the
            # last gather's buffer was fully overwritten after any in-place
            # reduce of its parity, so its raw slabs are intact
            zg = zg_sbs[((n_iters - 2) // 2) % 2]
            nc.vector.tensor_tensor(
                zg[:, 0:4, 1, :], zg[:, 0:4, 1, :], zg[:, 4:8, 1, :], ADD
            )
            nc.vector.tensor_tensor(
                zg[:, 0:2, 1, :], zg[:, 0:2, 1, :], zg[:, 2:4, 1, :], ADD
            )
            nc.vector.scalar_tensor_tensor(
                zsum[:], zg[:, 0, 1, :], S, zg[:, 1, 1, :], ADD, ADD
            )
            nc.vector.reciprocal(zsum[:], zsum[:])
            nc.vector.tensor_tensor(bf_f[:], zsum[:], btS[:], MULT)
        # final AF in f32 from the last y (still in PSUM)
        for it in range(RT):
            nc.vector.tensor_scalar_add(t_rt[:, ds(it, 1)], y_pss[it][:], S)
        nc.vector.reciprocal(t_rt[:], t_rt[:])
        nc.vector.tensor_tensor(af_f[:], t_rt[:], atS[:], MULT)

        # ---- final: C = K * AF[:,None] * BF[None,:] -------------------------
        nc.tensor.transpose(tr_ps[0:JT, :], bf_f[:], ident_f32[:, :])
        nc.vector.tensor_copy(bf_row[:], tr_ps[0:JT, :])
        nc.sync.dma_start(bf_dram[:], bf_row[:])
        nc.sync.dma_start(
            bf_flat[:], bf_dram[:].rearrange("t p -> (t p)").unsqueeze(0)
        )
        nc.gpsimd.partition_broadcast(bf_bc[:], bf_flat[:])

        for it in range(RT):
            stg = stage_pool.tile([128, NB], F32, tag="stage")
            nc.sync.dma_start(stg[:], K_d[ts(it, 128), :])
            cst = stage_pool.tile([128, NB], F32, tag="cstage")
            nc.vector.scalar_tensor_tensor(
                cst[:], stg[:], af_f[:, ds(it, 1)], bf_bc[:], MULT, MULT
            )
            nc.sync.dma_start(C_d[ts(it, 128), :], cst[:])

    es.close()


_CACHE = {}


def _n_dev(n_iters: int) -> int:
    # n_iters is the reference (Gauss-Seidel) sweep count; the device runs a
    # two-iteration-stale Jacobi scheme which needs ~10% more sweeps to reach
    # the same fixed-point accuracy (and an even count for pair batching)
    n = n_iters + 10 if n_iters >= 20 else max(4, n_iters)
    return n + (n % 2)


def _get_program(n_iters: int = N_ITERS, variant: str = "main"):
    key = (_n_dev(n_iters), variant)
    if key not in _CACHE:
        _CACHE[key] = build_program(_n_dev(n_iters), variant)
    return _CACHE[key]


def kernel(AT, BT, K, n_iters: int = N_ITERS, trace: bool = False,
           variant: str = "main"):
    nc = _get_program(n_iters, variant)
    AT = np.ascontiguousarray(AT, dtype=np.float32)
    BT = np.ascontiguousarray(BT, dtype=np.float32)
    K = np.ascontiguousarray(K, dtype=np.float32)
    in_maps = [
        {"K": K[c * R : (c + 1) * R], "AT": AT[c * R : (c + 1) * R], "BT": BT}
        for c in range(NCORES)
    ]
    res = bass_utils.run_bass_kernel_spmd(
        nc, in_maps, core_ids=list(range(NCORES)), trace=trace
    )
    C = np.concatenate([res.results[c]["C"] for c in range(NCORES)], axis=0)
    if trace:
        kernel.last_results = res
    return C
